# revision 36
# baseline (speedup 1.0000x reference)
"""GQA multi-head attention (B=2, T=2048, C=2048, H=32, KVH=8, HD=64) with RoPE
and causal masking, distributed over 8 Trainium2 NeuronCores.

Sharding: core c -> batch b = c//4, head-group g = c%4 (8 Q-heads + 2 KV-heads
per core; Wq/Wk/Wv column-parallel, Wo row-parallel).  Each core computes a
full [T, C] partial of the output projection in bf16; the host sums the 4
partials of each batch.

v2 rewrite targeting the PE p-state ramp (2.4GHz needs continuous PE busy):
 - x is transposed on the HOST; x^T streams in bf16 directly (no on-chip
   transposes of x, which cost the baseline ~130us of PE+copy time).
 - All inputs bf16 (halves HBM traffic); weights DMA'd once and kept in SBUF.
 - Local head slots are ordered so head slot s uses partitions 64*(s%2) in
   both Q^T and K^T (even slots -> kv0, odd -> kv1), keeping score matmuls'
   stationary/moving partition bases equal.
 - Phase A (projections), C (attention) and D (out-proj) are emission-
   interleaved: A(sb+1)/D(sb) matmuls are injected between C(sb) score/AV
   chunks so the PE never starves while the Act engine runs exp.
 - Softmax denominator: ones column in V_aug; per (head, q-block) epilogue
   uses reciprocal_approx_fast (the baseline's full-precision RECIPROCAL on
   one partition cost 103us).
"""

import numpy as np

B, T, C = 2, 2048, 2048
H, KVH, HD = 32, 8, 64
NCORES = 8
QC = 512            # q columns per core (8 heads)
NT = T // 128       # 16 t/k tiles
NCP = C // 128      # 16 c panels
NSB = 4             # t superblocks
SBT = T // NSB      # 512
EXP_SCALE = 1.0 / np.sqrt(HD)


def _build_program():
    import concourse.bass as bass
    import concourse.mybir as mybir
    import concourse.tile as tile
    from collections import deque
    from concourse import bacc
    from concourse._compat import get_trn_type
    from concourse.masks import make_identity, make_upper_triangular

    F32 = mybir.dt.float32
    BF16 = mybir.dt.bfloat16
    MUL = mybir.AluOpType.mult
    ADD = mybir.AluOpType.add
    SUB = mybir.AluOpType.subtract

    nc = bacc.Bacc(get_trn_type() or "TRN2", target_bir_lowering=False, debug=True)

    xt = nc.dram_tensor("xt", [C, T], BF16, kind="ExternalInput")
    wq = nc.dram_tensor("wq", [C, QC], BF16, kind="ExternalInput")
    wkv = nc.dram_tensor("wkv", [C, 256], BF16, kind="ExternalInput")
    wo = nc.dram_tensor("wo", [QC, C], BF16, kind="ExternalInput")
    csj = nc.dram_tensor("csj", [T, 64], F32, kind="ExternalInput")  # [t, cos32|sin32]
    outp = nc.dram_tensor("outp", [T, C], BF16, kind="ExternalOutput")

    def tt_op(out, in0, in1, op):
        nc.vector.tensor_tensor(out=out, in0=in0, in1=in1, op=op)

    with tile.TileContext(nc) as tc:
        with (
            tc.tile_pool(name="p1", bufs=1) as p1,
            tc.tile_pool(name="p2", bufs=2) as p2,
            tc.tile_pool(name="p3", bufs=3) as p3,
            tc.tile_pool(name="psA", bufs=2, space="PSUM") as psA,
            tc.tile_pool(name="psKV", bufs=2, space="PSUM") as psKV,
            tc.tile_pool(name="psS", bufs=2, space="PSUM") as psS,
            tc.tile_pool(name="psO", bufs=2, space="PSUM") as psO,
        ):
            # ---- input DMAs first (constants below run on GpSimd and would
            # delay the DMA issues by ~14us) ----
            wkv_s = p1.tile([128, NCP, 256], BF16)
            wkv_r = wkv[:].rearrange("(n p) q -> p n q", p=128)
            nc.gpsimd.dma_start(wkv_s[:, 0:8, :], wkv_r[:, 0:8, :])
            nc.gpsimd.dma_start(wkv_s[:, 8:16, :], wkv_r[:, 8:16, :])
            csj_t = p1.tile([128, NT, 64], F32)
            nc.gpsimd.dma_start(csj_t[:], csj[:].rearrange("(n p) d -> p n d", p=128))
            wq_s = p1.tile([128, NCP, QC], BF16)
            nc.gpsimd.dma_start(wq_s[:], wq[:].rearrange("(n p) q -> p n q", p=128))
            wo_s = p1.tile([128, 4, C], BF16)  # DMA deferred into gen_A(1)

            # ---- constants ----
            id32 = p1.tile([128, 128], F32)
            make_identity(nc, id32[:])
            idb = p1.tile([128, 128], BF16)
            nc.vector.tensor_copy(idb[:], id32[:])
            mk32 = p1.tile([128, 128], F32)
            make_upper_triangular(nc, mk32[:], val=1.0, diag=True)
            maskT = p1.tile([128, 128], BF16)
            nc.vector.tensor_copy(maskT[:], mk32[:])

            # ---- persistent activations ----
            QT = p1.tile([128, 4, T], BF16)      # [2 slots x 64d, block, t]
            KT = p1.tile([128, T], BF16)         # [kv0 (e|o) | kv1 (e|o), t]
            KTr = p1.tile([128, T], BF16)        # pre-rope K^T
            VT = p1.tile([128, T], BF16)         # [kv0 d | kv1 d, t]
            Vaug = p1.tile([128, 2, NT, HD + 1], BF16)  # [t-in-tile, kv, ktile, d|1]
            ohT = p1.tile([128, 4, T], BF16)     # attention out, D layout

            onescol = p1.tile([128, 1], BF16)
            nc.gpsimd.memset(onescol[:], 1.0)
            oc = onescol[:]
            ones_b = bass.AP(oc.tensor, oc.offset, [oc.ap[0], [0, 2 * NT], [1, 1]])
            nc.vector.tensor_copy(Vaug[:, :, :, HD:HD + 1], ones_b)



            # ================= Phase A: projections + rope =================
            def gen_A(sb):
                t0 = sb * SBT
                xTs = p2.tile([128, NCP, SBT], BF16, tag="xts", name=f"xts{sb}")
                xt_r = xt[:, t0:t0 + SBT].rearrange("(n p) t -> p n t", p=128)
                for q in range(4):  # chunked so the first matmuls start sooner
                    nc.sync.dma_start(xTs[:, 4 * q:4 * q + 4, :], xt_r[:, 4 * q:4 * q + 4, :])
                if sb == 1:  # wo not needed before phase D; keep it off the head
                    nc.gpsimd.dma_start(wo_s[:], wo[:].rearrange("(m p) c -> p m c", p=128))
                yield
                # K^T / V^T panels (contract over C)
                pk = psKV.tile([128, SBT], F32, tag="kvtr", name=f"pk{sb}")
                pv = psKV.tile([128, SBT], F32, tag="kvtr", name=f"pv{sb}")
                for ci in range(NCP):  # K/V interleaved: matches DMA arrival rate
                    nc.tensor.matmul(pk[:], wkv_s[:, ci, 0:128], xTs[:, ci, :],
                                     start=(ci == 0), stop=(ci == NCP - 1))
                    yield
                    nc.tensor.matmul(pv[:], wkv_s[:, ci, 128:256], xTs[:, ci, :],
                                     start=(ci == 0), stop=(ci == NCP - 1))
                    yield
                nc.vector.tensor_copy(KTr[:, t0:t0 + SBT], pk[:])
                nc.vector.tensor_copy(VT[:, t0:t0 + SBT], pv[:])
                # Per t-tile, software-pipelined by one tile: Q proj matmuls of
                # tile tl hide the DVE rope chains; Q transposes of tile tl-1
                # run after tile tl's matmuls (their rope is long done).
                def emit_qtr(tl):
                    tt = sb * 4 + tl
                    qr = qr_tiles[tl]
                    trq = psKV.tile([128, QC], BF16, tag="kvtr", name=f"trq{sb}_{tl}")
                    for m in range(4):
                        nc.tensor.transpose(trq[:, m * 128:(m + 1) * 128],
                                            qr[:, m * 128:(m + 1) * 128], idb[:])
                    nc.vector.tensor_copy(QT[:, :, tt * 128:(tt + 1) * 128],
                                          trq[:].rearrange("p (m t) -> p m t", m=4))

                qr_tiles = {}
                for tl in range(NSB):
                    tt = sb * 4 + tl
                    k0 = tt * 128
                    # K natural tile + rope chain (DVE work queued early)
                    trk = psKV.tile([128, 128], BF16, tag="kvtr", name=f"trk{sb}_{tl}")
                    nc.tensor.transpose(trk[:], KTr[:, k0:k0 + 128], idb[:])
                    kn = p2.tile([128, 128], BF16, tag="kn")
                    nc.vector.tensor_copy(kn[:], trk[:])
                    knr = p2.tile([128, 128], BF16, tag="knr")
                    kn3 = kn[:].rearrange("p (v d) -> p v d", v=2)
                    knr3 = knr[:].rearrange("p (v d) -> p v d", v=2)
                    ke, ko = kn3[:, :, 0:32], kn3[:, :, 32:64]
                    kre, kro = knr3[:, :, 0:32], knr3[:, :, 32:64]
                    cj = csj_t[:, tt, 0:32]
                    sj = csj_t[:, tt, 32:64]
                    cjb2 = bass.AP(cj.tensor, cj.offset, [cj.ap[0], [0, 2], [1, 32]])
                    sjb2 = bass.AP(sj.tensor, sj.offset, [sj.ap[0], [0, 2], [1, 32]])
                    k1 = p2.tile([128, 64], F32, tag="k1")
                    k2 = p2.tile([128, 64], F32, tag="k2")
                    k13 = k1[:].rearrange("p (v d) -> p v d", v=2)
                    k23 = k2[:].rearrange("p (v d) -> p v d", v=2)
                    tt_op(k13, ke, cjb2, MUL)
                    tt_op(k23, ko, sjb2, MUL)
                    tt_op(kre, k13, k23, SUB)
                    k1b = p2.tile([128, 64], F32, tag="k1")
                    k2b = p2.tile([128, 64], F32, tag="k2")
                    k13b = k1b[:].rearrange("p (v d) -> p v d", v=2)
                    k23b = k2b[:].rearrange("p (v d) -> p v d", v=2)
                    tt_op(k13b, ko, cjb2, MUL)
                    tt_op(k23b, ke, sjb2, MUL)
                    tt_op(kro, k13b, k23b, ADD)
                    yield
                    # Q projection for this tile (PE-dense; hides DVE chains)
                    pq = psA.tile([128, QC], F32, tag="pqo", name=f"pq{sb}_{tl}")
                    for ci in range(NCP):
                        nc.tensor.matmul(pq[:], xTs[:, ci, tl * 128:(tl + 1) * 128],
                                         wq_s[:, ci, :],
                                         start=(ci == 0), stop=(ci == NCP - 1))
                        yield
                    qn = p2.tile([128, QC], F32, tag="qn")
                    nc.vector.tensor_copy(qn[:], pq[:])
                    qr = p2.tile([128, QC], BF16, tag="qr")
                    qr_tiles[tl] = qr
                    qn3 = qn[:].rearrange("p (s d) -> p s d", s=8)
                    qr3 = qr[:].rearrange("p (s d) -> p s d", s=8)
                    qe, qo = qn3[:, :, 0:32], qn3[:, :, 32:64]
                    qre, qro = qr3[:, :, 0:32], qr3[:, :, 32:64]
                    cjb = bass.AP(cj.tensor, cj.offset, [cj.ap[0], [0, 8], [1, 32]])
                    sjb = bass.AP(sj.tensor, sj.offset, [sj.ap[0], [0, 8], [1, 32]])
                    t1 = p2.tile([128, 256], F32, tag="t1")
                    t2 = p2.tile([128, 256], F32, tag="t2")
                    t13 = t1[:].rearrange("p (s d) -> p s d", s=8)
                    t23 = t2[:].rearrange("p (s d) -> p s d", s=8)
                    tt_op(t13, qe, cjb, MUL)
                    tt_op(t23, qo, sjb, MUL)
                    tt_op(qre, t13, t23, SUB)
                    t1b = p2.tile([128, 256], F32, tag="t1")
                    t2b = p2.tile([128, 256], F32, tag="t2")
                    t13b = t1b[:].rearrange("p (s d) -> p s d", s=8)
                    t23b = t2b[:].rearrange("p (s d) -> p s d", s=8)
                    tt_op(t13b, qo, cjb, MUL)
                    tt_op(t23b, qe, sjb, MUL)
                    tt_op(qro, t13b, t23b, ADD)
                    yield
                    if tl > 0:
                        emit_qtr(tl - 1)
                        yield
                    trk2 = psKV.tile([128, 128], BF16, tag="kvtr", name=f"trk2{sb}_{tl}")
                    nc.tensor.transpose(trk2[:], knr[:], idb[:])
                    nc.vector.tensor_copy(KT[:, k0:k0 + 128], trk2[:])
                    yield
                    trv = psKV.tile([128, 128], BF16, tag="kvtr", name=f"trv{sb}_{tl}")
                    nc.tensor.transpose(trv[:], VT[:, k0:k0 + 128], idb[:])
                    for kv in range(2):
                        nc.vector.tensor_copy(Vaug[:, kv, tt, 0:HD],
                                              trv[:, kv * 64:(kv + 1) * 64])
                    yield
                emit_qtr(NSB - 1)
                yield

            # ================= Phase D: output projection =================
            def gen_D(sb):
                for tl in range(NSB):
                    tt = sb * 4 + tl
                    ost = p2.tile([128, C], BF16, tag="ost")
                    for cc in range(4):
                        po = psA.tile([128, 512], F32, tag="pqo", name=f"po{sb}_{tl}_{cc}")
                        for m in range(4):
                            nc.tensor.matmul(po[:], ohT[:, m, tt * 128:(tt + 1) * 128],
                                             wo_s[:, m, cc * 512:(cc + 1) * 512],
                                             start=(m == 0), stop=(m == 3))
                            if m < 3:
                                yield
                        if (tl * 4 + cc) % 4 == 3:
                            nc.scalar.copy(ost[:, cc * 512:(cc + 1) * 512], po[:])
                        else:
                            nc.vector.tensor_copy(ost[:, cc * 512:(cc + 1) * 512], po[:])
                        if cc % 2 == 1:
                            nc.sync.dma_start(
                                outp[tt * 128:(tt + 1) * 128, (cc - 1) * 512:(cc + 1) * 512],
                                ost[:, (cc - 1) * 512:(cc + 1) * 512])
                        yield

            # ================= Phase C: attention =================
            bg = deque()

            def pump(n=1):
                for _ in range(n):
                    while bg:
                        try:
                            next(bg[0])
                            break
                        except StopIteration:
                            bg.popleft()

            def drain(gen):
                for _ in gen:
                    pass

            def emit_C(sb):
                q0 = sb * SBT
                nki = 4 * sb + 4
                for s in (1, 3, 5, 7, 0, 2, 4, 6):  # DMA-path slots first
                    qb = 64 * (s % 2)
                    souT = psO.tile([HD + 1, SBT], F32, tag="sou", name=f"sou{sb}_{s}")
                    for ki in range(nki):
                        k0 = ki * 128
                        g0 = max(k0, q0)
                        w = q0 + SBT - g0
                        ps = psS.tile([128, SBT], F32, tag="sS", name=f"ps{sb}_{s}_{ki}")
                        nc.tensor.matmul(ps[:, 0:w], KT[qb:qb + 64, k0:k0 + 128],
                                         QT[qb:qb + 64, s // 2, g0:g0 + w],
                                         start=True, stop=True)
                        ptile = p3.tile([128, SBT], BF16, tag="pt", bufs=4)
                        nc.scalar.activation(ptile[:, 0:w], ps[:, 0:w],
                                             mybir.ActivationFunctionType.Exp,
                                             scale=float(EXP_SCALE))
                        if ki >= 4 * sb:  # diagonal tile: mask q<k after exp
                            tt_op(ptile[:, 0:128], ptile[:, 0:128], maskT[:], MUL)
                        nc.tensor.matmul(souT[:, g0 - q0:SBT], Vaug[:, s % 2, ki, :],
                                         ptile[:, 0:w],
                                         start=(ki == 0), stop=(ki == nki - 1))
                        # drain-priority for A gens; deficit-matched rate for D
                        pump((4 if sb == 0 else 3) if bg and bg[0] in a_set else 1)
                    # epilogue: divide by denominator (row 64 of souT).  The
                    # copy realigns to partition 0 — custom-DVE ops cannot
                    # read cross-partition (verified: garbage on HW).
                    rr = p2.tile([1, SBT], F32, tag="rr")
                    nc.vector.tensor_copy(rr[0:1, :], souT[HD:HD + 1, :])
                    rv = p2.tile([1, SBT], F32, tag="rv")
                    nc.vector.reciprocal_approx_fast(rv[0:1, :], rr[0:1, :])
                    rp = p2.tile([64, SBT], F32, tag="rp")
                    nc.gpsimd.partition_broadcast(rp[:], rv[0:1, :], channels=64)
                    if s % 2 == 0:
                        tt_op(ohT[0:64, s // 2, q0:q0 + SBT], souT[0:HD, :], rp[:], MUL)
                    else:
                        stg = p2.tile([64, SBT], BF16, tag="stg")
                        tt_op(stg[:], souT[0:HD, :], rp[:], MUL)
                        nc.sync.dma_start(ohT[64:128, s // 2, q0:q0 + SBT], stg[:])
                    pump()

            gens_a = [gen_A(sb) for sb in range(NSB)]
            gens_d = [gen_D(sb) for sb in range(NSB)]
            a_set = set(gens_a)
            with nc.named_scope("phaseA0"):
                drain(gens_a[0])
            # D generators are held back so C3 (the largest attention block,
            # which has no A fillers left) still has PE filler work.
            d_sched = {2: [0], 3: [1, 2]}
            for sb in range(NSB):
                with nc.named_scope(f"phaseC{sb}"):
                    if sb > 0:
                        drain(gens_a[sb])  # force-finish A(sb) before C(sb)
                    if sb < NSB - 1:
                        bg.appendleft(gens_a[sb + 1])  # A fillers before D fillers
                    for di in d_sched.get(sb, []):
                        bg.append(gens_d[di])
                    emit_C(sb)
            with nc.named_scope("phaseTail"):
                bg.append(gens_d[3])
                while bg:
                    try:
                        next(bg[0])
                    except StopIteration:
                        bg.popleft()

    nc.finalize()
    return nc


_RUNNER = None


def _get_runner():
    """Build the program once and return a cached jitted 8-core runner."""
    global _RUNNER
    if _RUNNER is not None:
        return _RUNNER

    import jax
    import concourse.mybir as mybir
    from concourse import bass2jax
    from jax.experimental.shard_map import shard_map
    from jax.sharding import Mesh, PartitionSpec

    nc = _build_program()
    bass2jax.install_neuronx_cc_hook()

    partition_name = nc.partition_id_tensor.name if nc.partition_id_tensor else None
    in_names, out_names, out_avals, zero_outs = [], [], [], []
    for alloc in nc.m.functions[0].allocations:
        if not isinstance(alloc, mybir.MemoryLocationSet):
            continue
        name = alloc.memorylocations[0].name
        if alloc.kind == "ExternalInput":
            if name != partition_name:
                in_names.append(name)
        elif alloc.kind == "ExternalOutput":
            shape = tuple(alloc.tensor_shape)
            dtype = mybir.dt.np(alloc.dtype)
            out_names.append(name)
            out_avals.append(jax.core.ShapedArray(shape, dtype))
            zero_outs.append(np.zeros(shape, dtype))
    n_params = len(in_names)
    n_outs = len(out_avals)
    all_names = list(in_names) + list(out_names)
    if partition_name is not None:
        all_names.append(partition_name)
    donate = tuple(range(n_params, n_params + n_outs))

    def _body(*args):
        operands = list(args)
        if partition_name is not None:
            operands.append(bass2jax.partition_id_tensor())
        outs = bass2jax._bass_exec_p.bind(
            *operands,
            out_avals=tuple(out_avals),
            in_names=tuple(all_names),
            out_names=tuple(out_names),
            lowering_input_output_aliases=(),
            sim_require_finite=True,
            sim_require_nnan=True,
            nc=nc,
        )
        return tuple(outs)

    devices = jax.devices()[:NCORES]
    mesh = Mesh(np.asarray(devices), ("core",))
    sharded = jax.jit(
        shard_map(_body, mesh=mesh,
                  in_specs=(PartitionSpec("core"),) * (n_params + n_outs),
                  out_specs=(PartitionSpec("core"),) * n_outs,
                  check_rep=False),
        donate_argnums=donate, keep_unused=True,
    )

    def run(in_maps):
        if nc.dbg_addr is not None:
            # No BassDebugger under axon; a zero PA makes the debug guard skip.
            dbg = np.zeros((1, 2), np.uint32)
            in_maps = [{**m, nc.dbg_addr.name: dbg} for m in in_maps]
        concat_in = [
            np.concatenate([np.asarray(in_maps[c][name]) for c in range(NCORES)], axis=0)
            for name in in_names
        ]
        concat_zeros = [np.zeros((NCORES * z.shape[0], *z.shape[1:]), z.dtype)
                        for z in zero_outs]
        out_arrs = sharded(*concat_in, *concat_zeros)
        return [
            {name: np.asarray(out_arrs[i]).reshape(NCORES, *out_avals[i].shape)[c]
             for i, name in enumerate(out_names)}
            for c in range(NCORES)
        ]

    _RUNNER = run
    return run


def make_in_maps(x, freq_cis, Wq, Wk, Wv, Wo):
    """Host-side sharding: per-core input dicts (all heavy tensors in bf16)."""
    import ml_dtypes
    bf16 = ml_dtypes.bfloat16

    x = np.asarray(x, np.float32)
    freq_cis = np.asarray(freq_cis, np.float32)
    Wq, Wk, Wv, Wo = (np.asarray(a, np.float32) for a in (Wq, Wk, Wv, Wo))

    cos, sin = freq_cis[:, :, 0], freq_cis[:, :, 1]            # [T, 32]
    csj = np.ascontiguousarray(np.concatenate([cos, sin], axis=1))  # [T, 64]

    dperm = np.concatenate([np.arange(0, HD, 2), np.arange(1, HD, 2)])  # evens|odds
    xts = [np.ascontiguousarray(x[b].T.astype(bf16)) for b in range(B)]
    in_maps = []
    for c in range(NCORES):
        b, g = divmod(c, 4)
        # head slot s -> global head: even slots from kv0's 4 heads,
        # odd slots from kv1's 4 heads (partition base 64*(s%2) everywhere)
        gheads = [g * 8 + (s // 2) + 4 * (s % 2) for s in range(8)]
        qcols = np.concatenate([gh * HD + dperm for gh in gheads])
        kcols = np.concatenate([(2 * g + kv) * HD + dperm for kv in range(2)])
        vcols = np.arange(2 * g * HD, (2 * g + 2) * HD)
        worows = np.concatenate([gh * HD + np.arange(HD) for gh in gheads])
        in_maps.append({
            "xt": xts[b],
            "wq": np.ascontiguousarray(Wq[:, qcols].astype(bf16)),
            "wkv": np.ascontiguousarray(
                np.concatenate([Wk[:, kcols], Wv[:, vcols]], axis=1).astype(bf16)),
            "wo": np.ascontiguousarray(Wo[worows, :].astype(bf16)),
            "csj": csj,
        })
    return in_maps


def combine_outputs(results):
    """Sum the 4 row-parallel bf16 partials of each batch."""
    out = np.zeros((B, T, C), np.float32)
    for c in range(NCORES):
        b = c // 4
        out[b] += np.asarray(results[c]["outp"]).astype(np.float32)
    return out


def kernel(x, freq_cis, mask, window, Wq, Wk, Wv, Wo):
    run = _get_runner()
    in_maps = make_in_maps(x, freq_cis, Wq, Wk, Wv, Wo)
    results = run(in_maps)
    return combine_outputs(results)
